# revision 125
# baseline (speedup 1.0000x reference)
"""Causal bilinear self-attention kernel for 8 Trainium2 NeuronCores.

Sharding: core c handles batch b = c//4 and head group g = c%4 (4 of 16
heads, feature slice [256g, 256g+256)).  Each core computes its partial
output-projection contribution y_partial = z_slice @ Wproj[:, slice].T and
the host sums the 4 partials per batch.

Projections run in fp8 (e4m3) DoubleRow mode at 0.5 cycles/row: x and the
qkv weights are split hi/lo on the host (w scaled by WSC=32 so the lo half
clears the e4m3 subnormal floor) and accumulated as hh + hl + lh — the
dropped ll term is ~0.13%, better than bf16.  The WSC and 1/HEAD_DIM
scales fold into the rope tables (q side) and the v drain.  Scores/z/proj
stay bf16; PSUM accumulation fp32.

RoPE is applied on-chip: the projection PSUM tile drains to SBUF (Act),
the sin-product of partner rows is 4 partition-offset muls against a
source-aligned signed sin table (DVE), cos-product and final add alternate
DVE/Pool.

z is accumulated in [q, d] orientation (out free = 64, half the cost of
streaming q): all 8 (head, q-subtile) chains share ONE PSUM bank, which is
engine-memzeroed first so every chain matmul can use start=False
(accumulate-onto-zero is correct in any execution order; a bank-first
start=True races under the PE's 32-deep OOO window).  The drained z is
PE-transposed (identity matmul) into the [feat, t] layout for C.

Schedule: phase A t-half 0 runs the q projection chunk-major (4 concurrent
PSUM accumulators) so PE starts when the first x piece lands (DMA is split
across SP/Act queues in need-order; HWDGE serializes ~640ns per DMA).
t-half 1 units drip into the B/C zips.  Phase B zips the two head-pair
blocks per kk round: scores (2+2 tile_position matmuls), Act S-drain to
bf16, DVE mul vs S2-in-PSUM (hw allows only one PSUM operand), DVE mask on
diagonal chunks, then the z chain matmuls.  C(qq) folds into the B(qq+1)
zip as extra drip work.
"""

import numpy as np
import ml_dtypes

import concourse.tile as tile
from concourse import bacc, mybir
from concourse.bass_utils import run_bass_kernel_spmd

D_MODEL = 1024
N_HEAD = 16
HEAD_DIM = 64  # Dh
B, T = 2, 2048
ROPE_BASE = 10000.0
N_CORES = 8
HG = 4          # heads per core
FS = HG * HEAD_DIM  # 256 features per core

F32 = mybir.dt.float32
BF16 = mybir.dt.bfloat16
F8 = mybir.dt.float8e4
NP_BF16 = ml_dtypes.bfloat16
NP_F8 = ml_dtypes.float8_e4m3
WSC = 32.0  # weight pre-scale: keeps the fp8 lo-half above the subnormal floor
DR = mybir.MatmulPerfMode.DoubleRow
TERMS = ((0, 0), (0, 1), (1, 0))  # (x half, W half): hh + hl + lh, drop ll

# (dst_row_start, src_row_start) pairs of the 32-row rope partner shuffle
SHUF = ((0, 32), (32, 0), (64, 96), (96, 64))

_PROGRAM = None

# schedule knobs: drips per zip round for qq=0..3, per C step, C2-insert
# cadence, and rv = how many early rope units run their cos-mul/add on DVE
KNOBS = {"d0": 9, "d1": 3, "dc": 2, "d2": 4, "ee": 2, "vtail": 1, "rv": 2,
         "alpha": 9, "e0": 2, "e1": 3, "mp": 0, "ce": 1, "tq": 0,
         "dw": 0, "du": 0, "bar": 72, "fp": 3, "ef": 4, "zd": 1,
         "pq": 14, "pt": 12, "pp": 12, "dm": 0, "pc": 10, "pm": 10, "pz": 4,
         "py": 8, "z2": 0}


def _build_program():
    nc = bacc.Bacc("TRN2", target_bir_lowering=False, debug=False)

    # fp8 hi/lo operands, consolidated so few large DMAs cover everything:
    # x: [half, chunk, 128, (slot, t)]; qk weights: [half, 128, (slot, ti, f, m)];
    # v weights: [128, (slot, half, n)]
    xt8_d = nc.dram_tensor("xt8", [2, 4, 128, 2, 2048], F8, kind="ExternalInput").ap()
    wqk8_d = nc.dram_tensor("wqk8", [2, 128, 8, 1024], F8, kind="ExternalInput").ap()
    wv8_d = nc.dram_tensor("wv8", [128, 8, 512], F8, kind="ExternalInput").ap()
    wpj_d = nc.dram_tensor("wpj", [128, 2048], BF16, kind="ExternalInput").ap()
    tabs_d = nc.dram_tensor("tabs", [4, 128, 2048], BF16, kind="ExternalInput").ap()
    masks_d = nc.dram_tensor("masks", [128, 2048], BF16, kind="ExternalInput").ap()
    idt_d = nc.dram_tensor("idt", [128, 128], BF16, kind="ExternalInput").ap()
    y_d = nc.dram_tensor("y", [T, D_MODEL], BF16, kind="ExternalOutput").ap()

    with tile.TileContext(nc) as tc:
        with (
            tc.tile_pool(name="pers", bufs=1) as pers,
            tc.tile_pool(name="xp", bufs=8) as xp,
            tc.tile_pool(name="wp", bufs=2) as wp,
            tc.tile_pool(name="wvp", bufs=1) as wvp,
            tc.tile_pool(name="mkp", bufs=1) as mkp,
            tc.tile_pool(name="qsp", bufs=KNOBS["pq"]) as qsp,
            tc.tile_pool(name="qcp", bufs=KNOBS["pc"]) as qcp,
            tc.tile_pool(name="tmp", bufs=KNOBS["pm"]) as tmp,
            tc.tile_pool(name="tsp", bufs=KNOBS["pt"]) as tsp,
            tc.tile_pool(name="zqp", bufs=KNOBS["pz"]) as zqp,
            tc.tile_pool(name="ysb", bufs=KNOBS["py"]) as ysb,
            tc.tile_pool(name="ptp", bufs=KNOBS["pp"]) as ptp,
            tc.tile_pool(name="psA", bufs=2, space="PSUM") as psA,
            tc.tile_pool(name="psS", bufs=2, space="PSUM") as psS,
            tc.tile_pool(name="psS2", bufs=2, space="PSUM") as psS2,
            tc.tile_pool(name="psZ", bufs=2, space="PSUM") as psZ,
        ):
            # persistent tiles
            proj = [pers.tile([128, T], BF16, tag=f"proj{i}", name=f"proj{i}")
                    for i in range(8)]
            # proj[2*ti+f] = chunk f of tensor ti (0=q,1=k,2=q2,3=k2)
            vt = [pers.tile([128, 1024], BF16, tag=f"v{i}", name=f"v{i}")
                  for i in range(4)]
            zt = [pers.tile([128, T], BF16, tag=f"z{i}", name=f"z{i}")
                  for i in range(2)]
            # separate rope tables for the q side (carries 1/(WSC*HEAD_DIM))
            # and the k side (carries 1/WSC)
            ctabs = [pers.tile([128, 2048], BF16, tag=f"ctab{i}", name=f"ctab{i}")
                     for i in range(2)]
            stabxs = [pers.tile([128, 2048], BF16, tag=f"stabx{i}", name=f"stabx{i}")
                      for i in range(2)]
            masks = mkp.tile([128, 2048], BF16, tag="masks")
            wpjt = pers.tile([128, 2048], BF16, tag="wpjt")
            idt = pers.tile([128, 128], BF16, tag="idt")
            if KNOBS["dw"]:
                zrt = pers.tile([128, 512], BF16, tag="zrt", name="zrt")
            else:
                zrt = None

            wqkt = [None, None]                            # [h] -> [128, 8, 1024]
            wvt = [None]                                   # [128, 8, 512]
            x8 = [[None] * 4 for _ in range(2)]            # [h][c] -> [128, 2, 2048]

            rope_flip = [0]

            def rope_consume(ps, ti, f, col):
                """Drain + RoPE a [128,512] projection PSUM tile into
                proj[2*ti+f][:, col:col+512].  The cos-mul and final add
                alternate between Pool and DVE so neither queue backs up."""
                tb = ti & 1  # 0 = q-side tables, 1 = k-side tables
                qs = qsp.tile([128, 512], BF16, tag="qs", name="qs")
                nc.scalar.copy(qs[:], ps[:])
                eng = nc.vector if rope_flip[0] < KNOBS["rv"] else nc.gpsimd
                rope_flip[0] += 1
                qc = qcp.tile([128, 512], BF16, tag="qc", name="qc")
                eng.tensor_mul(qc[:], qs[:], ctabs[tb][:, col:col + 512])
                tm = tmp.tile([128, 512], BF16, tag="tm", name="tm")
                for ds, ss in SHUF:
                    nc.vector.tensor_mul(
                        tm[ds:ds + 32, :], qs[ss:ss + 32, :],
                        stabxs[tb][ss:ss + 32, col:col + 512],
                    )
                eng.tensor_add(
                    proj[2 * ti + f][:, col:col + 512], qc[:], tm[:]
                )

            def qk_mms(ps, ti, f, tt, th):
                wcol = ti * 256 + f * 128
                xcol = th * 1024 + tt * 512
                n = 0
                for hx, hw in TERMS:
                    for c in range(4):
                        nc.tensor.matmul(
                            ps[:],
                            wqkt[hw][:, 2 * c:2 * c + 2, wcol:wcol + 128],
                            x8[hx][c][:, :, xcol:xcol + 512],
                            start=(n == 0),
                            stop=(n == 11),
                            perf_mode=DR,
                        )
                        n += 1

            def qk_unit(ti, f, tt, th):
                """One projection unit: 12 fp8 DoubleRow matmuls (3 hi/lo
                terms x 4 contraction chunks of 256)."""
                ps = psA.tile([128, 512], F32, tag="psa", name="psa")
                qk_mms(ps, ti, f, tt, th)
                rope_consume(ps, ti, f, th * 1024 + tt * 512)

            def v_mms(psv, m, th):
                xcol = th * 1024 + m * 128
                n = 0
                for hx, hw in TERMS:
                    for c in range(4):
                        nc.tensor.matmul(
                            psv[:, 0:256],
                            x8[hx][c][:, :, xcol:xcol + 128],
                            wvt[0][:, 2 * c:2 * c + 2, hw * 256:hw * 256 + 256],
                            start=(n == 0),
                            stop=(n == 11),
                            perf_mode=DR,
                        )
                        n += 1

            def v_unit(m, th):
                tg = th * 8 + m
                psv = psA.tile([128, 512], F32, tag="psa", name="psa")
                v_mms(psv, m, th)
                nc.scalar.mul(
                    vt[tg // 4][:, (tg % 4) * 256: (tg % 4) * 256 + 256],
                    psv[:, 0:256],
                    1.0 / WSC,
                )

            dum_state = {"ps": None, "open": False}

            def dum(n):
                """Emit n zero matmuls into the heater accumulator: they add
                exact zeros, cost N columns each, and keep the PE p-state
                ramp hot across phase-A DMA stalls."""
                if not KNOBS["dw"]:
                    return
                ps = dum_state["ps"]
                for _ in range(n):
                    nc.tensor.matmul(
                        ps[:], zrt[:, 0:128], zrt[:],
                        start=not dum_state["open"], stop=False,
                        skip_group_check=True,
                    )
                    dum_state["open"] = True

            def dum_close():
                if dum_state["open"]:
                    nc.tensor.matmul(
                        dum_state["ps"][:], zrt[:, 0:128], zrt[:],
                        start=False, stop=True, skip_group_check=True,
                    )
                    dum_state["open"] = False

            def emit_A0():
                # SP queue: the phase-A critical stream, split so the first
                # matmuls only wait on small pieces.  Act queue: tables,
                # v/proj weights, masks (HWDGE interleaves both queues).
                if KNOBS["dw"]:
                    nc.gpsimd.memset(zrt[:], 0.0)
                    dum_state["ps"] = psZ.tile(
                        [128, 512], F32, tag="zps", name="dps"
                    )
                    dum(4)
                for h in range(2):
                    t = wp.tile([128, 8, 1024], F8, tag="wqkt", name="wqkt")
                    wqkt[h] = t
                for h in range(2):
                    for c in range(4):
                        t = xp.tile([128, 2, 2048], F8, tag="x8", name="x8")
                        x8[h][c] = t

                # only the t-half-0 columns of the rope tables ride the
                # critical DMA window; the th1 halves come later
                nc.scalar.dma_start(ctabs[0][:, 0:1024], tabs_d[0][:, 0:1024])
                nc.scalar.dma_start(stabxs[0][:, 0:1024], tabs_d[1][:, 0:1024])
                nc.sync.dma_start(wqkt[0][:, 0:2, :], wqk8_d[0][:, 0:2, :])
                nc.sync.dma_start(x8[0][0][:, :, 0:1024], xt8_d[0][0][:, :, 0:1024])
                nc.sync.dma_start(wqkt[0][:, 2:4, :], wqk8_d[0][:, 2:4, :])
                nc.sync.dma_start(x8[0][1][:, :, 0:1024], xt8_d[0][1][:, :, 0:1024])
                nc.sync.dma_start(wqkt[0][:, 4:8, :], wqk8_d[0][:, 4:8, :])
                nc.sync.dma_start(x8[0][2][:, :, 0:1024], xt8_d[0][2][:, :, 0:1024])
                nc.sync.dma_start(x8[0][3][:, :, 0:1024], xt8_d[0][3][:, :, 0:1024])
                nc.scalar.dma_start(ctabs[1][:, 0:1024], tabs_d[2][:, 0:1024])
                nc.scalar.dma_start(stabxs[1][:, 0:1024], tabs_d[3][:, 0:1024])
                nc.sync.dma_start(wqkt[1][:], wqk8_d[1])
                for c in range(4):
                    nc.sync.dma_start(
                        x8[1][c][:, :, 0:1024], xt8_d[1][c][:, :, 0:1024]
                    )
                t = wvp.tile([128, 8, 512], F8, tag="wvt", name="wvt")
                nc.scalar.dma_start(t[:], wv8_d[:])
                wvt[0] = t
                nc.scalar.dma_start(masks[:], masks_d)
                nc.scalar.dma_start(idt[:], idt_d)

                # chunk-major q projection: 4 concurrent accumulators so the
                # PE's 32-deep OOO window always has DMA-ready work
                qpools = [(psA, "psa"), (psS, "sps"), (psS2, "s2ps"), (psZ, "zps")]
                units = [(0, 0), (1, 0), (0, 1), (1, 1)]  # (f, tt)
                qps = [pool.tile([128, 512], F32, tag=tag, name="qps")
                       for pool, tag in qpools]
                n = 0
                for hx, hw in TERMS:
                    for c in range(4):
                        for u, (f, tt) in enumerate(units):
                            nc.tensor.matmul(
                                qps[u][:],
                                wqkt[hw][:, 2 * c:2 * c + 2, f * 128:f * 128 + 128],
                                x8[hx][c][:, :, tt * 512: tt * 512 + 512],
                                start=(n == 0),
                                stop=(n == 11),
                                perf_mode=DR,
                            )
                        n += 1
                        dum(KNOBS["dw"])
                # (tt, f)-major so each B block's proj dependencies clear the
                # Pool/Act queues in block order
                for u, (f, tt) in enumerate(units):
                    rope_consume(qps[u], 0, f, tt * 512)
                    for ti in range(1, 4):
                        qk_unit(ti, f, tt, 0)
                        dum(KNOBS["du"])
                for m in range(4):
                    v_unit(m, 0)
                dum_close()

                # queue t-half-1 x, table halves and the output weight behind
                # the phase-A critical stream on the SP queue — putting them
                # on the Act queue would block the rope drains behind ~25us
                # of HWDGE issue serialization
                for h in range(2):
                    for c in range(4):
                        nc.sync.dma_start(
                            x8[h][c][:, :, 1024:2048], xt8_d[h][c][:, :, 1024:2048]
                        )
                for i in range(2):
                    nc.sync.dma_start(ctabs[i][:, 1024:2048], tabs_d[2 * i][:, 1024:2048])
                    nc.sync.dma_start(
                        stabxs[i][:, 1024:2048], tabs_d[2 * i + 1][:, 1024:2048]
                    )
                nc.sync.dma_start(wpjt[:], wpj_d[:])

            def qk_unit_gen(ti, f, tt, th):
                """qk_unit split at op granularity so interleaving into B
                blocks never head-of-line-blocks a latency-critical op."""
                tb = ti & 1
                ps = psA.tile([128, 512], F32, tag="psa", name="psa")
                qk_mms(ps, ti, f, tt, th)
                yield
                col = th * 1024 + tt * 512
                qs = qsp.tile([128, 512], BF16, tag="qs", name="qs")
                nc.scalar.copy(qs[:], ps[:])
                yield
                eng = nc.gpsimd
                qc = qcp.tile([128, 512], BF16, tag="qc", name="qc")
                eng.tensor_mul(qc[:], qs[:], ctabs[tb][:, col:col + 512])
                yield
                tm = tmp.tile([128, 512], BF16, tag="tm", name="tm")
                for ds, ss in SHUF:
                    nc.vector.tensor_mul(
                        tm[ds:ds + 32, :], qs[ss:ss + 32, :],
                        stabxs[tb][ss:ss + 32, col:col + 512],
                    )
                    yield
                eng.tensor_add(
                    proj[2 * ti + f][:, col:col + 512], qc[:], tm[:]
                )
                yield

            def v_unit_gen(m, th):
                tg = th * 8 + m
                psv = psA.tile([128, 512], F32, tag="psa", name="psa")
                v_mms(psv, m, th)
                yield
                nc.scalar.mul(
                    vt[tg // 4][:, (tg % 4) * 256: (tg % 4) * 256 + 256],
                    psv[:, 0:256],
                    1.0 / WSC,
                )
                yield

            def emit_A1():
                for tt in range(2):
                    for f in range(2):
                        for ti in range(4):
                            yield from qk_unit_gen(ti, f, tt, 1)
                    if tt == 0:
                        for m in range(4):
                            yield from v_unit_gen(m, 1)
                if not KNOBS["vtail"]:
                    for m in range(4, 8):
                        yield from v_unit_gen(m, 1)

            def emit_v1_tail():
                # vt[3] is only read by B(3) kk>=12: defer these units into
                # the B(3) zip as PE filler
                for m in range(4, 8):
                    yield from v_unit_gen(m, 1)

            def drain(gen, n):
                for _ in range(n):
                    try:
                        next(gen)
                    except StopIteration:
                        return False
                return True

            def scores_kk(qq, hp, kk):
                off = max(0, kk - 4 * qq) * 128
                qsl = slice(qq * 512 + off, qq * 512 + 512)
                ksl = slice(kk * 128, kk * 128 + 128)
                kT, qT = proj[2 + hp], proj[0 + hp]
                k2T, q2T = proj[6 + hp], proj[4 + hp]
                sps = [None, None]
                s2ps = [None, None]
                # hh-major so the hh0 bilinear mul's operands complete first
                for hh in range(2):
                    rb = 64 * hh
                    sp = psS.tile([128, 512], F32, tag="sps", name="sps")
                    nc.tensor.matmul(
                        sp[:, off:512], kT[rb:rb + 64, ksl], qT[rb:rb + 64, qsl],
                        start=True, stop=True, tile_position=(rb, 0),
                    )
                    sps[hh] = sp
                    s2 = psS2.tile([128, 512], F32, tag="s2ps", name="s2ps")
                    nc.tensor.matmul(
                        s2[:, off:512], k2T[rb:rb + 64, ksl], q2T[rb:rb + 64, qsl],
                        start=True, stop=True, tile_position=(rb, 0),
                    )
                    s2ps[hh] = s2
                return sps, s2ps

            def bilinear(qq, hp, kk, sps, s2ps):
                """Produce the masked bf16 pattern tiles for both heads.
                The DVE mul may read only ONE operand from PSUM (hw rule),
                so Act drains S to a bf16 SBUF tile and DVE multiplies it
                with the S2 PSUM tile directly."""
                off = max(0, kk - 4 * qq) * 128
                pts = []
                for hh in range(2):
                    ts = tsp.tile([128, 512], BF16, tag="ts", name="ts")
                    nc.scalar.copy(ts[:, off:512], sps[hh][:, off:512])
                    pt = ptp.tile([128, 512], BF16, tag="pt", name="pt")
                    nc.vector.tensor_mul(
                        pt[:, off:512], ts[:, off:512], s2ps[hh][:, off:512]
                    )
                    if kk >= 4 * qq:
                        j = kk - 4 * qq
                        meng = nc.gpsimd if KNOBS["mp"] else nc.vector
                        meng.tensor_mul(
                            pt[:, off:off + 128],
                            pt[:, off:off + 128],
                            masks[:, j * 512 + off: j * 512 + off + 128],
                        )
                    pts.append(pt)
                return pts

            def z_mms(qq, hp, kk, pts, zps, zfirst):
                """z in [q, d] orientation: out free = 64 instead of the
                512-wide q stream — halves the z matmul cost.  All 8 chains
                (hh, qs) pack into ONE PSUM bank.  The bank is memzeroed by
                an engine write first and every matmul uses start=False:
                accumulate-onto-zero is correct in ANY execution order (a
                bank-first start=True would race under the PE's OOO window
                and wipe sibling chains)."""
                j = kk - 4 * qq
                for hh in range(2):
                    vsl = (kk % 4) * 256 + (2 * hp + hh) * 64
                    for qs in range(4):
                        if qs < j:
                            continue
                        co = (4 * hh + qs) * 64
                        nc.tensor.matmul(
                            zps[:, co:co + 64],
                            pts[hh][:, qs * 128: qs * 128 + 128],
                            vt[kk // 4][:, vsl:vsl + 64],
                            start=False,
                            stop=(kk == 4 * qq + qs),
                            skip_group_check=True,
                        )

            def drain_zq(qq, hp, qs, zps, zq_s):
                """Tail variant: drain/transpose one q-subtile as soon as its
                chains stop, so C steps overlap the remaining B rounds."""
                for hh in range(2):
                    co = (4 * hh + qs) * 64
                    nc.scalar.copy(zq_s[:, co:co + 64], zps[:, co:co + 64])
                ztq = psS2.tile([128, 1024], BF16, tag="s2ps", name="ztq")
                for hh in range(2):
                    co = (4 * hh + qs) * 64
                    nc.tensor.matmul(
                        ztq[64 * hh:64 * hh + 64, 0:128],
                        zq_s[:, co:co + 64],
                        idt[:],
                        is_transpose=True,
                        start=(hh == 0),
                        stop=True,
                        skip_group_check=True,
                    )
                col = qq * 512 + qs * 128
                nc.scalar.copy(zt[hp][:, col:col + 128], ztq[:, 0:128])

            def drain_z(qq, hp, zps):
                """zps holds 8 [128 q, 64 d] blocks; drain to SBUF, PE-
                transpose each to [d, q], and drain into zt's [feat, t]
                layout for the output projection."""
                zq_s = zqp.tile([128, 512], BF16, tag="zqs", name="zqs")
                if KNOBS["zd"] and hp == 1:
                    # split the two blocks' z drains across Act and DVE
                    nc.vector.tensor_copy(zq_s[:], zps[:])
                else:
                    nc.scalar.copy(zq_s[:], zps[:])
                ztp = psS2.tile([128, 1024], BF16, tag="s2ps", name="ztp")
                # the 8 transposes share one source tile (zq_s) so they all
                # become ready together and execute in queue order: the
                # start=True first transpose cannot be overtaken (unlike the
                # z chains, whose cross-hh inputs finish at different times)
                first = True
                for hh in range(2):
                    for qs in range(4):
                        co = (4 * hh + qs) * 64
                        nc.tensor.matmul(
                            ztp[64 * hh:64 * hh + 64, qs * 128:qs * 128 + 128],
                            zq_s[:, co:co + 64],
                            idt[:],
                            is_transpose=True,
                            start=first,
                            stop=True,
                            skip_group_check=True,
                        )
                        first = False
                if KNOBS["z2"] and hp == 1:
                    nc.vector.tensor_copy(
                        zt[hp][:, qq * 512: qq * 512 + 512], ztp[:, 0:512]
                    )
                else:
                    nc.scalar.copy(
                        zt[hp][:, qq * 512: qq * 512 + 512], ztp[:, 0:512]
                    )



            yo_cur = [None]
            dma_queues = [nc.sync, nc.scalar]
            dma_flip = [0]

            def emit_C_step(qq, ypool, ytag, tg, oo, drain_eng="alt"):
                yps = ypool.tile([128, 512], F32, tag=ytag, name="yps")
                for ci in range(2):
                    nc.tensor.matmul(
                        yps[:],
                        zt[ci][:, tg * 128: tg * 128 + 128],
                        wpjt[:, ci * 1024 + oo * 512
                             : ci * 1024 + oo * 512 + 512],
                        start=(ci == 0),
                        stop=(ci == 1),
                    )
                if oo == 0:
                    yo_cur[0] = ysb.tile([128, 1024], BF16, tag="yo", name="yo")
                yo = yo_cur[0]
                if KNOBS["ce"]:
                    drain_eng = "act"
                use_act = drain_eng == "act" or (drain_eng == "alt" and (2 * tg + oo) % 2 == 0)
                if use_act:
                    nc.scalar.copy(yo[:, oo * 512: oo * 512 + 512], yps[:])
                else:
                    nc.vector.tensor_copy(yo[:, oo * 512: oo * 512 + 512], yps[:])
                if oo == 1:
                    nc.sync.dma_start(y_d[tg * 128: tg * 128 + 128, :], yo[:])

            def emit_C(qq, ypool, ytag):
                for tg in range(4 * qq, 4 * qq + 4):
                    for oo in range(2):
                        emit_C_step(qq, ypool, ytag, tg, oo)

            def emit_C_gen(qq, ypool, ytag, drain_eng="alt"):
                for tg in range(4 * qq, 4 * qq + 4):
                    for oo in range(2):
                        emit_C_step(qq, ypool, ytag, tg, oo, drain_eng)
                        yield

            # ---------------- emission schedule ----------------
            emit_A0()

            def emit_tail_v0():
                for m in range(4, 8):
                    yield from v_unit_gen(m, 0)

            import itertools
            gen1_cnt = [0]

            def counted(g):
                for x in g:
                    gen1_cnt[0] += 1
                    yield x

            gen1 = counted(itertools.chain(emit_tail_v0(), emit_A1()))

            def zip_blocks(qq, drip, front=None, front_per=0, extra=None,
                           extra_every=0, extra_from=0, tailq=False):
                last = 4 * qq + 3
                zA = psZ.tile([128, 512], F32, tag="zps", name="zA")
                zB = psZ.tile([128, 512], F32, tag="zps", name="zB")
                zts = (zA, zB)
                nc.scalar.memzero(zA[:])
                nc.scalar.memzero(zB[:])
                zqs_t = [None, None]

                def tail_piece(qs):
                    for hp in range(2):
                        if zqs_t[hp] is None:
                            zqs_t[hp] = zqp.tile(
                                [128, 512], BF16, tag="zqs", name="zqs"
                            )
                        drain_zq(qq, hp, qs, zts[hp], zqs_t[hp])
                    for oo in range(2):
                        emit_C_step(qq, psA, "psa", 4 * qq + qs, oo)

                for kk in range(last + 1):
                    for hp in range(2):
                        sps, s2ps = scores_kk(qq, hp, kk)
                        pts = bilinear(qq, hp, kk, sps, s2ps)
                        z_mms(qq, hp, kk, pts, zts[hp], None)
                        if hp == 0 and drip and KNOBS["dm"]:
                            drain(gen1, KNOBS["dm"])
                    if tailq and 0 <= kk - 4 * qq - 1 <= 2:
                        tail_piece(kk - 4 * qq - 1)
                    if drip:
                        drain(gen1, drip)
                    if front is not None and front_per:
                        drain(front, front_per)
                    if (extra is not None and extra_every and kk >= extra_from
                            and kk % extra_every == extra_every - 1):
                        drain(extra, 1)
                if tailq:
                    tail_piece(3)
                else:
                    for hp in range(2):
                        drain_z(qq, hp, zts[hp])

            # C(qq) folds into the B(qq+1) zip as extra work so no serial
            # C window (and its drain-z latency) ever exposes PE idle time
            zip_blocks(0, drip=KNOBS["d0"])
            gc0 = emit_C_gen(0, psA, "psa")
            zip_blocks(1, drip=KNOBS["d1"], extra=gc0, extra_every=KNOBS["e0"])
            drain(gc0, 1000)
            gc1 = emit_C_gen(1, psA, "psa")
            # barrier: B(2) kk=0 reads the tt0 t-half-1 q/q2 projections
            # (gen1 steps <= 64); if the drips haven't emitted them yet the
            # scores read unwritten SBUF (NaN on hw — the cost-model sim
            # doesn't execute values so only hw catches it).  The k-side
            # (steps <= 72) and vt[2] (<= 80) are needed from kk=8, covered
            # by the in-zip drips when bar + 8*d2 >= 80.
            drain(gen1, max(0, KNOBS["bar"] - gen1_cnt[0]))
            zip_blocks(2, drip=KNOBS["d2"], extra=gc1, extra_every=KNOBS["e1"])
            drain(gen1, 1000)  # flush remaining A(1) units
            drain(gc1, 1000)
            gc2 = emit_C_gen(2, psA, "psa", drain_eng="act")
            tq = KNOBS["tq"]
            if KNOBS["vtail"]:
                # deferred vt[3] units MUST be fully emitted well before the
                # B(3) rounds that read them (PE queue order = execution order)
                gv = emit_v1_tail()
                zip_blocks(3, drip=0, front=gv, front_per=KNOBS["fp"],
                           extra=gc2, extra_every=KNOBS["ee"],
                           extra_from=KNOBS["ef"],
                           tailq=tq)
                drain(gv, 1000)
            else:
                zip_blocks(3, drip=0, extra=gc2, extra_every=KNOBS["ee"],
                           tailq=tq)
            drain(gc2, 1000)
            if not tq:
                emit_C(3, psA, "psa")

    nc.compile()
    return nc


def _get_program():
    global _PROGRAM
    if _PROGRAM is None:
        _PROGRAM = _build_program()
    return _PROGRAM


def _hilo(a):
    """Split float64 array into fp8 hi + lo parts."""
    hi = a.astype(NP_F8)
    lo = (a - hi.astype(np.float64)).astype(NP_F8)
    return hi, lo


def _pack_wqk8(wlist):
    """wlist: 4 matrices [256 out, 1024 in], pre-scaled by WSC.  Returns
    [2 h, 128, 8, 1024] fp8: row p, (s, ti*256 + f*128 + m) =
    wlist[ti].T[128*s + p, 128*f + m]."""
    out = np.empty((2, 128, 8, 1024), dtype=NP_F8)
    for ti, ws in enumerate(wlist):
        a = np.ascontiguousarray(ws.T).astype(np.float64)  # [1024, 256]
        for h, m8 in enumerate(_hilo(a)):
            r = m8.reshape(8, 128, 2, 128)  # (s, p, f, m)
            out[h, :, :, ti * 256:(ti + 1) * 256] = (
                r.transpose(1, 0, 2, 3).reshape(128, 8, 256)
            )
    return out


def _pack_wv8(ws):
    """ws: [256 out, 1024 in] pre-scaled.  [128, 8, 512]: (s, h*256+n)."""
    a = np.ascontiguousarray(ws.T).astype(np.float64)  # [1024, 256]
    out = np.empty((128, 8, 512), dtype=NP_F8)
    for h, m8 in enumerate(_hilo(a)):
        r = m8.reshape(8, 128, 256)  # (s, p, n)
        out[:, :, h * 256:(h + 1) * 256] = r.transpose(1, 0, 2)
    return out


def _pack_x8(x_b):
    """x_b: [T, D] float.  Returns [2 h, 4 c, 128, 2, 2048] fp8 where
    row p, (i, t) = x_b.T[256*c + 128*i + p, t]."""
    a = np.ascontiguousarray(x_b.T).astype(np.float64)  # [1024, 2048]
    out = np.empty((2, 4, 128, 2, 2048), dtype=NP_F8)
    for h, m8 in enumerate(_hilo(a)):
        r = m8.reshape(4, 2, 128, 2048)  # (c, i, p, t)
        out[h] = r.transpose(0, 2, 1, 3)
    return out


def _make_tabs():
    inv = 1.0 / (ROPE_BASE ** (np.arange(0, HEAD_DIM, 2, dtype=np.float32) / HEAD_DIM))
    t = np.arange(T, dtype=np.float32)
    ang = np.outer(t, inv)  # [T, 32]
    c32 = np.cos(ang).astype(np.float32).T  # [32, T]
    s32 = np.sin(ang).astype(np.float32).T
    ctab = np.tile(c32, (4, 1))  # [128, T]
    # source-aligned signed sin table: the shuffle op reads src rows ss:ss+32
    # of both the drained q tile and this table, writing dst rows ds:ds+32.
    # dst 0:32 needs +sin (src rows 32:64), dst 32:64 needs -sin (src 0:32).
    stabx = np.concatenate([-s32, s32, -s32, s32], axis=0)  # [128, T]
    # q-side tables also fold in the 1/HEAD_DIM score scale; both sides fold
    # out the WSC weight pre-scale
    qs, ks = 1.0 / (WSC * HEAD_DIM), 1.0 / WSC
    tabs = np.ascontiguousarray(
        np.stack([ctab * qs, stabx * qs, ctab * ks, stabx * ks])
    ).astype(NP_BF16)
    r = np.arange(128)[:, None]
    ccol = np.arange(512)[None, :]
    masks = np.ascontiguousarray(np.concatenate(
        [(ccol >= r + 128 * j).astype(np.float32) for j in range(4)], axis=1
    )).astype(NP_BF16)  # [128, 2048]
    return tabs, masks


def kernel(x, Wq, Wk, Wq2, Wk2, Wv, Wproj):
    x = np.asarray(x, dtype=np.float32)
    Wq = np.asarray(Wq, dtype=np.float32)
    Wk = np.asarray(Wk, dtype=np.float32)
    Wq2 = np.asarray(Wq2, dtype=np.float32)
    Wk2 = np.asarray(Wk2, dtype=np.float32)
    Wv = np.asarray(Wv, dtype=np.float32)
    Wproj = np.asarray(Wproj, dtype=np.float32)

    nc = _get_program()
    tabs, masks = _make_tabs()
    idt = np.eye(128, dtype=NP_BF16)

    xt8 = [_pack_x8(x[b]) for b in range(B)]
    in_maps = []
    for c in range(N_CORES):
        b, g = divmod(c, HG)
        fsl = slice(g * FS, g * FS + FS)
        wqk8 = np.ascontiguousarray(_pack_wqk8(
            [Wq[fsl] * WSC, Wk[fsl] * WSC, Wq2[fsl] * WSC, Wk2[fsl] * WSC]
        ))
        wv8 = _pack_wv8(Wv[fsl] * WSC)
        wpj = np.ascontiguousarray(
            Wproj[:, fsl].T.reshape(2, 128, 1024).transpose(1, 0, 2).reshape(128, 2048)
        ).astype(NP_BF16)
        in_maps.append({"xt8": xt8[b], "wqk8": wqk8, "wv8": wv8, "wpj": wpj,
                        "tabs": tabs, "masks": masks, "idt": idt})

    res = run_bass_kernel_spmd(nc, in_maps, list(range(N_CORES))).results

    y = np.zeros((B, T, D_MODEL), dtype=np.float64)
    for c in range(N_CORES):
        b = c // HG
        y[b] += np.asarray(res[c]["y"]).astype(np.float64)
    return y.astype(np.float32)

